# revision 17
# baseline (speedup 1.0000x reference)
import sys

sys.path.insert(0, "/opt/trn_rl_repo")

import numpy as np

import concourse.bass as bass
import concourse.mybir as mybir
import concourse.tile as tile
from concourse import bacc
from concourse.ap import AP
from concourse.bass_utils import run_bass_kernel_spmd

F32 = mybir.dt.float32
F16 = mybir.dt.float16
BF16 = mybir.dt.bfloat16
FP8 = mybir.dt.float8e4
AFT = mybir.ActivationFunctionType
DR = mybir.MatmulPerfMode.DoubleRow
MULT = mybir.AluOpType.mult

CIN, H, W = 64, 256, 256
COUT, KHW = 128, 3
HO, WO = 254, 254
NCORES = 8
NI = 17
NPLANES = 4
NWSLOT = 11
P_PLANE = NI * W

CHUNKS = [0, 32, 64, 96, 128, 160, 192, 222]

S_X = 2.0
S_W = 16.0
S_MM = S_X * S_W

HD = -234.90094743115216
HA = -1.170775735238817e-14
HB = 3.172370771708323e-09
HC = -0.0003325704427109515
H0 = 10.100263990836655

_CUSTOM_OPS = {}


def _register_custom_ops():
    if _CUSTOM_OPS:
        return _CUSTOM_OPS
    import re as _re

    import concourse.dve_ops as dv
    from concourse.dve_spec import (Spec, Src0, C0, C1, C2, C3,
                                    _spill_c3_to_src1)

    _x = Src0 + C0
    _s = _x * _x
    body = _spill_c3_to_src1(((C1 * _s + C2) * _s + C3) * _s)

    def _ref(in0, in1, c0, c1, c2):
        c3 = in1
        if isinstance(c3, np.ndarray):
            c3 = c3.reshape(c3.shape[0], 1)
            in0 = in0.reshape(in0.shape[0], -1)
        x = in0 + c0
        s = x * x
        return ((c1 * s + c2) * s + c3) * s

    name = "CV_H"
    spec = Spec(body=body, reference=_ref)
    op = dv.DveOp(name, spec, subdim=False, uops_sha={})
    if name not in dv._SUB_OPCODE_FOR_NAME:
        dv._SUB_OPCODE_FOR_NAME[name] = max(dv._SUB_OPCODE_FOR_NAME.values()) + 1
    try:
        op.compile("v3")
    except ValueError as e:
        m = _re.search(r'uops_sha\["v3"\]="([0-9a-f]+)"', str(e))
        assert m, f"no sha in: {e}"
        op = dv.DveOp(name, spec, subdim=False, uops_sha={"v3": m.group(1)})
    op.compile("v3")
    dv.OPS.append(op)
    dv.CUSTOM_DVE_SPECS[name] = spec
    _CUSTOM_OPS["H"] = op
    return _CUSTOM_OPS


K64_SLOT = 9


def even_instrs(i):
    return [
        (0, 0, i, 0, 2),
        (1, 0, i + 1, 0, 2),
        (2, 0, i, 1, W),
        (3, 1, i, 0, 2),
        (4, 2, i, 0, 2),
        (5, 1, i, 1, P_PLANE),
        (6, 0, i + 1, 2, 3 * P_PLANE - 2),
    ]


def odd_instrs(i):
    return [
        (3, 0, i + 1, 0, 2),
        (0, 1, i, 0, 2),
        (1, 1, i + 1, 0, 2),
        (2, 1, i, 1, W),
        (7, 2, i + 1, 0, 2),
        (8, 0, i + 1, 1, 2 * P_PLANE),
        (9, 1, i, 0, 2),
        (10, 1, i, 1, 2),
    ]


def build_nc(n_prime=6, ew_pat=None, debug_mode=None):
    ops = _register_custom_ops()
    OPH = ops["H"]
    nc = bacc.Bacc("TRN2", target_bir_lowering=False, debug=False, num_devices=1)

    xp_d = nc.dram_tensor("xp", [128, NPLANES, H // 2, W], FP8,
                          kind="ExternalInput")
    wm_d = nc.dram_tensor("wm", [128, NWSLOT, 2, COUT], FP8,
                          kind="ExternalInput")
    y_d = nc.dram_tensor("y", [COUT, HO, WO], BF16, kind="ExternalOutput")
    y_ap = y_d.ap()

    if ew_pat is None:
        ew_pat = ["psum" if (g % 8 in (1, 4, 6) or g % 32 == 3) else "pool"
                  for g in range(64)]
        ew_pat[-3:] = ["psum"] * 3

    with tile.TileContext(nc) as tc:
        with (
            tc.tile_pool(name="wpool", bufs=1) as wpool,
            tc.tile_pool(name="xpool", bufs=3) as xpool,
            tc.tile_pool(name="ppool", bufs=4, space="PSUM") as ppool,
            tc.tile_pool(name="hpool", bufs=6) as hpool,
            tc.tile_pool(name="opool", bufs=6) as opool,
        ):
            wt = wpool.tile([128, NWSLOT, 2, COUT], FP8, tag="wt")
            nc.sync.dma_start(wt[:], wm_d.ap())

            hc = wpool.tile([128, 1], F32, tag="hc")
            nc.vector.memset(hc[:], HC)
            h0t = wpool.tile([128, 1], F32, tag="h0")
            nc.vector.memset(h0t[:], H0)

            def load_chunk(ro0):
                xt = xpool.tile([128, NPLANES, NI, W], FP8, tag="xt",
                                name=f"xt{ro0}")
                i0 = ro0 // 2
                for (a, b) in ((0, 5), (5, 9), (9, 13), (13, NI)):
                    nc.sync.dma_start(
                        xt[:, :, a:b, :], xp_d.ap()[:, :, i0 + a:i0 + b, :])
                return xt

            xt0 = load_chunk(0)

            warm = wpool.tile([128, 1], F16, tag="warm")
            nc.scalar.activation(warm[:], h0t[:], AFT.Sigmoid, bias=h0t[:])
            nc.scalar.activation(warm[:], warm[:], AFT.Identity)

            if n_prime:
                prime = ppool.tile([128, 2, 512], F32, tag="pg", name="prime")
                for _ in range(n_prime):
                    nc.tensor.matmul(
                        prime[:, 0, 0:256], wt[:, 0, 0, :],
                        wt[:, 0, :, :].rearrange("p a c -> p (a c)"),
                        start=True, stop=True,
                    )

            def rhs_ap(xt, wslot, plane, drow, col, delta):
                a = xt[:]
                off = a.offset + plane * P_PLANE + drow * W + col
                pdim = list(a.ap[0])
                if wslot == K64_SLOT:
                    pdim = [pdim[0], 64]
                return AP(a.tensor, off,
                          [pdim, [delta, 2], [W, 2], [1, 254]])

            loaded = {0: xt0}
            g_all = 0
            for ci, ro0 in enumerate(CHUNKS):
                for cj in (ci + 1, ci + 2):
                    if cj < len(CHUNKS) and cj not in loaded:
                        loaded[cj] = load_chunk(CHUNKS[cj])
                xt = loaded.pop(ci)

                for g in range(8):
                    r0g = ro0 + 4 * g
                    ib = (r0g - ro0) // 2
                    pg = ppool.tile([128, 2, 512], F32, tag="pg",
                                    name=f"pg{r0g}")
                    for b in range(2):
                        instrs = even_instrs(ib) if b == 0 else odd_instrs(ib)
                        n = len(instrs)
                        for k, (ws, pl, drw, cl, dlt) in enumerate(instrs):
                            lhsT = wt[0:64, ws, :, :] if ws == K64_SLOT \
                                else wt[:, ws, :, :]
                            nc.tensor.matmul(
                                pg[:, b, 0:508],
                                lhsT,
                                rhs_ap(xt, ws, pl, drw, cl, dlt),
                                start=(k == 0), stop=(k == n - 1),
                                perf_mode=DR,
                            )

                    mode = ew_pat[g_all % len(ew_pat)]
                    g_all += 1

                    pin = pg[:, :, 0:508]
                    outt = opool.tile([128, 4 * WO], BF16, tag="outt",
                                      name=f"ot{r0g}")
                    oa = outt[:]
                    dst = AP(oa.tensor, oa.offset,
                             [list(oa.ap[0]), [WO, 2], [2 * WO, 2], [1, WO]])

                    def ew_ap(t):
                        a = t[:]
                        return AP(a.tensor, a.offset,
                                  [list(a.ap[0]), [512, 2], [254, 2],
                                   [1, 254]])

                    if debug_mode == "raw":
                        nc.vector.tensor_copy(dst, ew_ap(pg))
                        nc.sync.dma_start(y_ap[:, r0g:r0g + 4, :], outt[:])
                        continue

                    ht = hpool.tile([128, 2, 512], F16, tag="ht",
                                    name=f"ht{r0g}")
                    nc.vector._custom_dve(
                        OPH, out=ht[:, :, 0:508], in0=pin, in1=hc[:],
                        s0=HD, s1=HA, imm2=HB,
                    )
                    sg = hpool.tile([128, 2, 512], F16, tag="sg",
                                    name=f"sg{r0g}")
                    nc.scalar.activation(sg[:, :, 0:508], ht[:, :, 0:508],
                                         AFT.Sigmoid, bias=h0t[:])

                    if mode == "psum":
                        nc.vector.tensor_tensor(dst, ew_ap(pg), ew_ap(sg),
                                                MULT)
                    else:
                        yb = hpool.tile([128, 2, 512], F16, tag="yb",
                                        name=f"yb{r0g}")
                        nc.scalar.activation(yb[:, :, 0:508], pin,
                                             AFT.Identity)
                        _mul = nc.gpsimd if mode == "pool" else nc.vector
                        _mul.tensor_tensor(dst, ew_ap(yb), ew_ap(sg), MULT)

                    nc.sync.dma_start(y_ap[:, r0g:r0g + 4, :], outt[:])

    nc.compile()
    return nc


def pack_inputs(x, weight, bias_v):
    import ml_dtypes

    E4 = ml_dtypes.float8_e4m3
    x = np.ascontiguousarray(np.asarray(x, dtype=np.float32))
    weight = np.ascontiguousarray(np.asarray(weight, dtype=np.float32))
    bias_v = np.ascontiguousarray(np.asarray(bias_v, dtype=np.float32))

    whi = (S_W * weight).astype(E4)
    wlo = (S_W * weight - whi.astype(np.float32)).astype(E4)
    wb8 = (S_MM * bias_v).astype(E4)

    def tr(a):
        return np.ascontiguousarray(a.T)

    def A_slot(kh, kw):
        lo = tr(whi[:, :, kh, kw])
        up = lo.copy()
        up[0] = 0
        return lo, up

    def L_slot(kha, khb, kw):
        return tr(wlo[:, :, kha, kw]), tr(wlo[:, :, khb, kw])

    Z64 = np.zeros((64, COUT), dtype=E4)

    def PAD_slot(kh, kw):
        up = Z64.copy()
        up[0] = wb8
        return tr(wlo[:, :, kh, kw]), up

    A = {(kh, kw): A_slot(kh, kw) for kh in range(3) for kw in range(3)}

    def cat(pair):
        return np.concatenate(pair, axis=0)

    slots = [None] * NWSLOT
    slots[0] = (cat(A[0, 0]), cat(A[0, 2]))
    slots[1] = (cat(A[2, 0]), cat(A[2, 2]))
    slots[2] = (cat(A[0, 1]), cat(A[2, 1]))
    slots[3] = (cat(A[1, 0]), cat(A[1, 2]))
    slots[4] = (cat(L_slot(0, 1, 0)), cat(L_slot(0, 1, 2)))
    slots[5] = (cat(A[1, 1]), cat(L_slot(0, 1, 1)))
    slots[6] = (cat(PAD_slot(2, 2)),
                np.concatenate([tr(wlo[:, :, 2, 0]), tr(wlo[:, :, 2, 1])],
                               axis=0))
    slots[7] = (cat(L_slot(1, 2, 0)), cat(L_slot(1, 2, 2)))
    slots[8] = (cat(A[1, 1]), cat(L_slot(1, 2, 1)))
    k64_0 = np.concatenate([tr(wlo[:, :, 0, 0]), Z64], axis=0)
    k64_1 = np.concatenate([tr(wlo[:, :, 0, 2]), Z64], axis=0)
    slots[K64_SLOT] = (k64_0, k64_1)
    slots[10] = (cat(PAD_slot(0, 1)), np.zeros((128, COUT), dtype=E4))

    wm = np.zeros((128, NWSLOT, 2, COUT), dtype=E4)
    for si, (k0, k1) in enumerate(slots):
        wm[:, si, 0, :] = k0
        wm[:, si, 1, :] = k1

    in_maps = []
    for n in range(x.shape[0]):
        xn = x[n]
        xhi = (S_X * xn).astype(E4)
        xlo = (S_X * xn - xhi.astype(np.float32)).astype(E4)
        xlo = np.ascontiguousarray(xlo)
        xlo[0] = 1.0
        he = np.ascontiguousarray(xhi[:, 0::2, :])
        ho = np.ascontiguousarray(xhi[:, 1::2, :])
        le = np.ascontiguousarray(xlo[:, 0::2, :])
        lo_ = np.ascontiguousarray(xlo[:, 1::2, :])
        hes = np.zeros_like(he)
        hes[:, :, :W - 1] = he[:, :, 1:]
        xp = np.empty((128, NPLANES, H // 2, W), dtype=E4)
        xp[0:64, 0] = he
        xp[64:128, 0] = le
        xp[0:64, 1] = ho
        xp[64:128, 1] = lo_
        xp[0:64, 2] = he
        xp[64:128, 2] = ho
        xp[0:64, 3] = he
        xp[64:128, 3] = hes
        in_maps.append({"xp": xp, "wm": wm})
    return in_maps


_NC_CACHE = {}


def _get_nc():
    if "nc" not in _NC_CACHE:
        _NC_CACHE["nc"] = build_nc()
    return _NC_CACHE["nc"]


def kernel(x, weight, bias):
    nc = _get_nc()
    in_maps = pack_inputs(x, weight, bias)
    res = run_bass_kernel_spmd(nc, in_maps, core_ids=list(range(NCORES)))
    y = np.stack(
        [np.asarray(res.results[n]["y"]).astype(np.float32)
         for n in range(NCORES)], axis=0)
    return y * np.float32(1.0 / S_MM)
